# revision 1
# baseline (speedup 1.0000x reference)
"""GQA (16 q-heads / 4 kv-heads, D=128, S=2048, E=2048, B=2) on 8 trn2 cores.

Sharding: core = 4*b + g  (b in {0,1} batch, g in {0..3} kv-head group).
Each core computes its batch's 4 query heads (one kv group) end-to-end:
  QT/KT/VT projections (transposed layout, d on partitions), RoPE in
  transposed layout, scoresT = K @ Q^T per sk-tile, exp (no max subtraction:
  |scores*scale| <~ 6 for this input distribution), softmax denominator via
  DVE accumulation + ones-matmul partition reduce, AV with V-natural
  stationary producing outT, normalization by reciprocal broadcast
  (outer-product matmul), then o_proj with the group's wo row-block.
Host sums the 4 partial o_proj outputs per batch.

All matmuls run in float32r (full PE rate at N>=256 on TRN2).
"""

import numpy as np

import concourse.bass as bass
import concourse.bacc as bacc
import concourse.mybir as mybir
import concourse.tile as tile
from concourse.bass_utils import run_bass_kernel_spmd

B, S, E = 2, 2048, 2048
H, HKV, D = 16, 4, 128
G = H // HKV          # 4 query heads per kv group
GD = G * D            # 512 channels per group
NCORES = 8
SCALE = 1.0 / float(np.sqrt(D))
ROPE_BASE = 10000.0

NE = E // 128         # 16 e-chunks (contraction for projections)
NSC = S // 512        # 4 s-chunks of 512
NST = S // 128        # 16 s-tiles of 128

F32 = mybir.dt.float32
F32R = mybir.dt.float32r
AF = mybir.ActivationFunctionType
OP = mybir.AluOpType


def _r(ap):
    return ap.bitcast(F32R)


def _emit(nc, tc, xT, wq, wk, wv, wo, cosT, sinTf, ident, onesd, out):
    from contextlib import ExitStack
    es = ExitStack()
    with es:
        cpool = es.enter_context(tc.tile_pool(name="const", bufs=1))
        qtpool = es.enter_context(tc.tile_pool(name="qt", bufs=1))

        # ---- always-live tiles ----
        id_sb = cpool.tile([128, 128], F32, tag="id")
        ones_sb = cpool.tile([128, 128], F32R, tag="ones")
        nc.sync.dma_start(out=id_sb[:], in_=ident.ap())
        nc.sync.dma_start(out=ones_sb[:], in_=onesd.ap().bitcast(F32R))

        qt_sb = [qtpool.tile([D, S], F32R, tag=f"qt{i}", name=f"qt{i}") for i in range(G)]
        kt_sb = cpool.tile([D, S], F32R, tag="kt")
        vn_sb = cpool.tile([128, NST, D], F32R, tag="vn")

        # ================= phase A: projections + RoPE =================
        with (
            tc.tile_pool(name="phA", bufs=1) as pa,
            tc.tile_pool(name="xs", bufs=16) as xpool,
            tc.tile_pool(name="ropetmp", bufs=2) as rpool,
            tc.tile_pool(name="psA", bufs=1, space=bass.MemorySpace.PSUM) as psA,
        ):
            wq_sb = [pa.tile([128, GD], F32R, tag=f"wq{j}", name=f"wq{j}")
                     for j in range(NE)]
            for j in range(NE):
                nc.sync.dma_start(out=wq_sb[j][:],
                                  in_=wq.ap()[j * 128:(j + 1) * 128, :].bitcast(F32R))
            cos_sb = pa.tile([D, S], F32, tag="cos")
            sin_sb = pa.tile([D, S], F32, tag="sin")
            nc.sync.dma_start(out=cos_sb[:], in_=cosT.ap())
            nc.sync.dma_start(out=sin_sb[:], in_=sinTf.ap())
            wk_sb = [pa.tile([128, D], F32R, tag=f"wk{j}", name=f"wk{j}")
                     for j in range(NE)]
            wv_sb = [pa.tile([128, D], F32R, tag=f"wv{j}", name=f"wv{j}")
                     for j in range(NE)]
            for j in range(NE):
                nc.sync.dma_start(out=wk_sb[j][:],
                                  in_=wk.ap()[j * 128:(j + 1) * 128, :].bitcast(F32R))
                nc.sync.dma_start(out=wv_sb[j][:],
                                  in_=wv.ap()[j * 128:(j + 1) * 128, :].bitcast(F32R))
            vt_sb = pa.tile([D, S], F32, tag="vt")

            def rope(dst_ap, ps, csl, ssl):
                # DVE lanes can't cross partitions: do rotate_half's partition
                # swap with two SBUF->SBUF DMAs, then aligned elementwise ops.
                qraw = rpool.tile([128, 512], F32, tag="qraw")
                qswp = rpool.tile([128, 512], F32, tag="qswp")
                rot = rpool.tile([128, 512], F32, tag="rot")
                tmc = rpool.tile([128, 512], F32, tag="tmc")
                nc.vector.tensor_copy(qraw[:], ps[:])
                nc.sync.dma_start(out=qswp[0:64, :], in_=qraw[64:128, :])
                nc.sync.dma_start(out=qswp[64:128, :], in_=qraw[0:64, :])
                nc.gpsimd.tensor_tensor(rot[:], qswp[:], ssl, OP.mult)
                nc.gpsimd.tensor_tensor(tmc[:], qraw[:], csl, OP.mult)
                nc.gpsimd.tensor_tensor(dst_ap, tmc[:], rot[:], OP.add)

            for q in range(NSC):
                sl = slice(q * 512, (q + 1) * 512)
                xsl = [xpool.tile([128, 512], F32R, tag="xs", name=f"xs{q}_{j}")
                       for j in range(NE)]
                for j in range(NE):
                    nc.sync.dma_start(out=xsl[j][:],
                                      in_=xT.ap()[j * 128:(j + 1) * 128, sl].bitcast(F32R))
                for h in range(G):
                    ps = psA.tile([128, 512], F32, tag="proj", bufs=3)
                    for j in range(NE):
                        nc.tensor.matmul(ps[:], _r(wq_sb[j][:, h * D:(h + 1) * D]),
                                         _r(xsl[j][:]), start=(j == 0), stop=(j == NE - 1))
                    rope(qt_sb[h][:, sl], ps, cos_sb[:, sl], sin_sb[:, sl])
                # K
                ps = psA.tile([128, 512], F32, tag="proj", bufs=3)
                for j in range(NE):
                    nc.tensor.matmul(ps[:], _r(wk_sb[j][:]), _r(xsl[j][:]),
                                     start=(j == 0), stop=(j == NE - 1))
                rope(kt_sb[:, sl], ps, cos_sb[:, sl], sin_sb[:, sl])
                # V (no rope) -> vt (transposed), converted to natural below
                ps = psA.tile([128, 512], F32, tag="proj", bufs=3)
                for j in range(NE):
                    nc.tensor.matmul(ps[:], _r(wv_sb[j][:]), _r(xsl[j][:]),
                                     start=(j == 0), stop=(j == NE - 1))
                nc.vector.tensor_copy(vt_sb[:, sl], ps[:])
                # V natural layout via PE transpose, interleaved per chunk
                for tt_ in range(4):
                    t = q * 4 + tt_
                    trp = psA.tile([128, 128], F32, tag="vtr", bufs=2)
                    nc.tensor.transpose(trp[:], vt_sb[:, t * 128:(t + 1) * 128], id_sb[:])
                    nc.vector.tensor_copy(vn_sb[:, t, :], trp[:])

        # ================= phase B: attention =================
        bcpool = es.enter_context(tc.tile_pool(name="phBC", bufs=1))
        wo_sb = [bcpool.tile([128, E], F32R, tag=f"wo{h}", name=f"wo{h}")
                 for h in range(G)]
        for h in range(G):
            nc.sync.dma_start(out=wo_sb[h][:],
                              in_=wo.ap()[h * 128:(h + 1) * 128, :].bitcast(F32R))
        ot_sb = [bcpool.tile([D, S], F32R, tag=f"ot{i}", name=f"ot{i}") for i in range(G)]
        with (
            tc.tile_pool(name="attn", bufs=8) as apool,
            tc.tile_pool(name="bwork", bufs=2) as bw,
            tc.tile_pool(name="psB", bufs=1, space=bass.MemorySpace.PSUM) as psB,
        ):
            for h in range(G):
                for q in range(NSC):
                    sl = slice(q * 512, (q + 1) * 512)
                    acc = bw.tile([128, 512], F32, tag="acc")
                    accp = bw.tile([128, 512], F32, tag="accp")
                    av = psB.tile([D, 512], F32, tag="av", bufs=2)
                    for t in range(NST):
                        sc = psB.tile([128, 512], F32, tag="sc", bufs=3)
                        nc.tensor.matmul(sc[:], _r(kt_sb[:, t * 128:(t + 1) * 128]),
                                         _r(qt_sb[h][:, sl]), start=True, stop=True)
                        at = apool.tile([128, 512], F32R, tag="attn")
                        nc.scalar.activation(at[:], sc[:], AF.Exp, scale=SCALE)
                        if t == 0:
                            nc.vector.tensor_copy(acc[:], at[:])
                        elif t < 10:
                            nc.vector.tensor_tensor(acc[:], acc[:], at[:], OP.add)
                        elif t == 10:
                            nc.gpsimd.tensor_copy(accp[:], at[:])
                        else:
                            nc.gpsimd.tensor_tensor(accp[:], accp[:], at[:], OP.add)
                        nc.tensor.matmul(av[:], _r(vn_sb[:, t, :]), _r(at[:]),
                                         start=(t == 0), stop=(t == NST - 1))
                    accm = bw.tile([128, 512], F32R, tag="accm")
                    nc.vector.tensor_tensor(accm[:], acc[:], accp[:], OP.add)
                    sm = psB.tile([1, 512], F32, tag="sm", bufs=1)
                    nc.tensor.matmul(sm[:], _r(ones_sb[:, 0:1]), _r(accm[:]),
                                     start=True, stop=True)
                    rc = bw.tile([1, 512], F32R, tag="rc")
                    with nc.allow_low_precision(reason="f32r softmax denominator, full fp32 bits"):
                        nc.vector.reciprocal(rc[:], sm[:])
                    bc = psB.tile([128, 512], F32, tag="bc", bufs=1)
                    nc.tensor.matmul(bc[:], _r(ones_sb[0:1, :]), _r(rc[:]),
                                     start=True, stop=True)
                    bcs = bw.tile([128, 512], F32, tag="bcs")
                    nc.vector.tensor_copy(bcs[:], bc[:])
                    nc.vector.tensor_tensor(ot_sb[h][:, sl], av[:], bcs[:], OP.mult)

        # ================= phase C: o_proj =================
        with (
            tc.tile_pool(name="ost", bufs=2) as opool,
            tc.tile_pool(name="psC", bufs=1, space=bass.MemorySpace.PSUM) as psC,
        ):
            for st in range(NST):
                ostg = opool.tile([128, E], F32, tag="ostg")
                for eo in range(4):
                    op_ps = psC.tile([128, 512], F32, tag="op", bufs=3)
                    for h in range(G):
                        nc.tensor.matmul(op_ps[:],
                                         _r(ot_sb[h][:, st * 128:(st + 1) * 128]),
                                         _r(wo_sb[h][:, eo * 512:(eo + 1) * 512]),
                                         start=(h == 0), stop=(h == G - 1))
                    nc.vector.tensor_copy(ostg[:, eo * 512:(eo + 1) * 512], op_ps[:])
                nc.sync.dma_start(out=out.ap()[st * 128:(st + 1) * 128, :], in_=ostg[:])


def _build():
    nc = bacc.Bacc("TRN2", target_bir_lowering=False, debug=False,
                   num_devices=NCORES)
    xT = nc.dram_tensor("xT", [E, S], F32, kind="ExternalInput")
    wq = nc.dram_tensor("wq", [E, GD], F32, kind="ExternalInput")
    wk = nc.dram_tensor("wk", [E, D], F32, kind="ExternalInput")
    wv = nc.dram_tensor("wv", [E, D], F32, kind="ExternalInput")
    wo = nc.dram_tensor("wo", [GD, E], F32, kind="ExternalInput")
    cosT = nc.dram_tensor("cosT", [D, S], F32, kind="ExternalInput")
    sinTf = nc.dram_tensor("sinTf", [D, S], F32, kind="ExternalInput")
    ident = nc.dram_tensor("ident", [128, 128], F32, kind="ExternalInput")
    onesd = nc.dram_tensor("onesd", [128, 128], F32, kind="ExternalInput")
    out = nc.dram_tensor("out", [S, E], F32, kind="ExternalOutput")
    with tile.TileContext(nc) as tc:
        _emit(nc, tc, xT, wq, wk, wv, wo, cosT, sinTf, ident, onesd, out)
    nc.compile()
    return nc


def _rope_tables():
    inv = 1.0 / (ROPE_BASE ** (np.arange(0, D, 2, dtype=np.float64) / D))
    t = np.arange(S, dtype=np.float64)
    freqs = t[:, None] * inv[None, :]                    # [S, D/2]
    emb = np.concatenate([freqs, freqs], axis=-1)        # [S, D]
    cosT = np.cos(emb).T.astype(np.float32)              # [D, S]
    sinT = np.sin(emb).T.astype(np.float32)
    sinTf = sinT.copy()
    sinTf[: D // 2] *= -1.0                              # fold rotate_half sign
    return np.ascontiguousarray(cosT), np.ascontiguousarray(sinTf)


_NC = None
LAST_RESULTS = None


def kernel(hidden_states, wq, wk, wv, wo):
    global _NC, LAST_RESULTS
    if _NC is None:
        _NC = _build()
    cosT, sinTf = _rope_tables()
    ident = np.eye(128, dtype=np.float32)
    hs = np.asarray(hidden_states, dtype=np.float32)
    wq = np.asarray(wq, dtype=np.float32)
    wk = np.asarray(wk, dtype=np.float32)
    wv = np.asarray(wv, dtype=np.float32)
    wo = np.asarray(wo, dtype=np.float32)

    in_maps = []
    for core in range(NCORES):
        b, g = divmod(core, G)
        in_maps.append({
            "xT": np.ascontiguousarray(hs[b].T),
            "wq": np.ascontiguousarray(wq[:, GD * g:GD * (g + 1)]),
            "wk": np.ascontiguousarray(wk[:, D * g:D * (g + 1)]),
            "wv": np.ascontiguousarray(wv[:, D * g:D * (g + 1)]),
            "wo": np.ascontiguousarray(wo[GD * g:GD * (g + 1), :]),
            "cosT": cosT,
            "sinTf": sinTf,
            "ident": ident,
            "onesd": np.ones((128, 128), dtype=np.float32),
        })

    res = run_bass_kernel_spmd(_NC, in_maps, list(range(NCORES)))
    LAST_RESULTS = res
    outs = [np.asarray(res.results[i]["out"], dtype=np.float32)
            for i in range(NCORES)]
    full = np.stack([sum(outs[b * G:(b + 1) * G]) for b in range(B)], axis=0)
    return full.astype(np.float32)



# revision 16
# speedup vs baseline: 1.0791x; 1.0791x over previous
"""GQA (16 q-heads / 4 kv-heads, D=128, S=2048, E=2048, B=2) on 8 trn2 cores.

Sharding: core = 4*b + g  (b in {0,1} batch, g in {0..3} kv-head group).
Each core computes its batch's 4 query heads (one kv group) end-to-end and
the host sums the 4 partial o_proj outputs per batch.

v2 layout (single interleaved program, engines balanced):
  - Phase A per 512-position chunk: load x e-blocks, K proj+RoPE, Q(h0)
    proj+RoPE, V proj + PE-transpose to natural bf16, Q(h1..h3) proj+RoPE.
    RoPE: rotate-half partition swap via 2 PSUM->SBUF DMAs, cos-mult on DVE,
    sin-mult on Pool (in-place over the swap tile), add on DVE.
  - Attention in two 1024-wide query chunks: scoresT per sk-tile in f32r,
    one wide [128,1024] Exp on the Act engine straight from PSUM to bf16
    SBUF, bf16 AV matmuls (V-natural stationary), softmax denominator via
    chained bf16 DVE adds + ones-matmul partition reduce, reciprocal,
    broadcast matmul, Pool multiply into ot (f32).
  - o_proj of chunk c interleaved into attention of chunk c+1; stores staged
    through SBUF via Pool copies.
"""

import numpy as np
import ml_dtypes

import concourse.bass as bass
import concourse.bacc as bacc
import concourse.mybir as mybir
import concourse.tile as tile
from concourse.bass_utils import run_bass_kernel_spmd

B, S, E = 2, 2048, 2048
H, HKV, D = 16, 4, 128
G = H // HKV          # 4 query heads per kv group
GD = G * D            # 512 channels per group
NCORES = 8
SCALE = 1.0 / float(np.sqrt(D))
ROPE_BASE = 10000.0

NE = E // 128         # 16 e-blocks (contraction for projections)
NC4 = S // 512        # 4 position chunks of 512 (projection granularity)
NST = S // 128        # 16 sk-tiles of 128
CH = 1024             # attention query-chunk width
NCH = S // CH         # 2 attention chunks

F32 = mybir.dt.float32
F32R = mybir.dt.float32r
BF16 = mybir.dt.bfloat16
AF = mybir.ActivationFunctionType
OP = mybir.AluOpType


def _r(ap):
    return ap.bitcast(F32R)


def _emit(nc, tc, xT, wq, wk, wv, wo, cosT, sinTf, ident, onesb, out):
    from contextlib import ExitStack
    es = ExitStack()
    with es:
        cpool = es.enter_context(tc.tile_pool(name="const", bufs=1))
        wpool = es.enter_context(tc.tile_pool(name="wts", bufs=1))
        wopool = es.enter_context(tc.tile_pool(name="wo", bufs=8))
        xpool = es.enter_context(tc.tile_pool(name="xs", bufs=16))
        rpool = es.enter_context(tc.tile_pool(name="rope", bufs=2))
        etpool = es.enter_context(tc.tile_pool(name="et", bufs=4))
        bcspool = es.enter_context(tc.tile_pool(name="bcs", bufs=1))
        dnpool = es.enter_context(tc.tile_pool(name="dn", bufs=2))
        rcpool = es.enter_context(tc.tile_pool(name="rc", bufs=1))
        otpool = es.enter_context(tc.tile_pool(name="ot", bufs=6))
        ostgpool = es.enter_context(tc.tile_pool(name="ostg", bufs=2))
        vtpool = es.enter_context(tc.tile_pool(name="vt", bufs=1))
        pssc = es.enter_context(
            tc.tile_pool(name="pssc", bufs=2, space=bass.MemorySpace.PSUM))
        psav = es.enter_context(
            tc.tile_pool(name="psav", bufs=1, space=bass.MemorySpace.PSUM))
        psmx = es.enter_context(
            tc.tile_pool(name="psmx", bufs=2, space=bass.MemorySpace.PSUM))

        # ---- constants ----
        id_sb = cpool.tile([128, 128], F32, tag="id")
        ones_sb = cpool.tile([128, 128], BF16, tag="ones")
        nc.sync.dma_start(out=id_sb[:], in_=ident.ap())
        nc.sync.dma_start(out=ones_sb[:], in_=onesb.ap())
        cos_sb = cpool.tile([D, S], BF16, tag="cos")
        sin_sb = cpool.tile([D, S], BF16, tag="sin")
        nc.sync.dma_start(out=cos_sb[:], in_=cosT.ap())
        nc.sync.dma_start(out=sin_sb[:], in_=sinTf.ap())

        # ---- weights (f32, bitcast to f32r at matmul) ----
        wq_sb = [wpool.tile([128, GD], F32R, tag=f"wq{j}", name=f"wq{j}")
                 for j in range(NE)]
        wk_sb = [wpool.tile([128, D], F32R, tag=f"wk{j}", name=f"wk{j}")
                 for j in range(NE)]
        wv_sb = [wpool.tile([128, D], F32R, tag=f"wv{j}", name=f"wv{j}")
                 for j in range(NE)]
        for j in range(NE):
            nc.sync.dma_start(out=wq_sb[j][:],
                              in_=wq.ap()[j * 128:(j + 1) * 128, :].bitcast(F32R))
            nc.sync.dma_start(out=wk_sb[j][:],
                              in_=wk.ap()[j * 128:(j + 1) * 128, :].bitcast(F32R))
            nc.sync.dma_start(out=wv_sb[j][:],
                              in_=wv.ap()[j * 128:(j + 1) * 128, :].bitcast(F32R))

        # ---- persistent activations ----
        kt = cpool.tile([D, S], F32R, tag="kt")
        qt = [cpool.tile([D, S], F32R, tag=f"qt{h}", name=f"qt{h}")
              for h in range(G)]
        vn = cpool.tile([128, NST, D], BF16, tag="vn")

        def rope(dst, ps, sl):
            # rotate_half partition swap via SBUF<->SBUF DMAs, then
            # dst = q*cos + swap*sin (sin sign-folded on host).
            qraw = rpool.tile([128, 512], F32, tag="qraw")
            qswp = rpool.tile([128, 512], F32, tag="qswp")
            tmc = rpool.tile([128, 512], F32, tag="tmc")
            nc.vector.tensor_copy(qraw[:], ps[:])
            nc.sync.dma_start(out=qswp[0:64, :], in_=qraw[64:128, :])
            nc.sync.dma_start(out=qswp[64:128, :], in_=qraw[0:64, :])
            nc.vector.tensor_tensor(tmc[:], qraw[:], cos_sb[:, sl], OP.mult)
            nc.gpsimd.tensor_tensor(qswp[:], qswp[:], sin_sb[:, sl], OP.mult)
            nc.vector.tensor_tensor(dst, tmc[:], qswp[:], OP.add)

        def proj(w_tiles, cols, xsl):
            assert cols == 128
            ps = psmx.tile([128, 512], F32, tag="mx", name="ps")
            for j in range(NE):
                nc.tensor.matmul(ps[:], _r(w_tiles[j][:, 0:cols]), _r(xsl[j][:]),
                                 start=(j == 0), stop=(j == NE - 1))
            return ps

        # ================= phase A: projections + RoPE =================
        for c4 in range(NC4):
            sl = slice(c4 * 512, (c4 + 1) * 512)
            xsl = [xpool.tile([128, 512], F32R, tag="xs", name=f"xs{c4}_{j}")
                   for j in range(NE)]
            for j in range(NE):
                nc.sync.dma_start(out=xsl[j][:],
                                  in_=xT.ap()[j * 128:(j + 1) * 128, sl].bitcast(F32R))
            # K
            ps = proj(wk_sb, D, xsl)
            rope(kt[:, sl], ps, sl)
            # Q head 0 first so attention can start as early as possible
            ps = proj([w[:, 0:D] for w in wq_sb], D, xsl)
            rope(qt[0][:, sl], ps, sl)
            # V -> natural bf16 via PE transpose
            ps = proj(wv_sb, D, xsl)
            vt = vtpool.tile([D, 512], F32, tag="vt")
            nc.scalar.copy(vt[:], ps[:])
            for tt in range(4):
                trp = psmx.tile([128, 128], F32, tag="mx", name="trp")
                nc.tensor.transpose(trp[:], vt[:, tt * 128:(tt + 1) * 128],
                                    id_sb[:])
                nc.vector.tensor_copy(vn[:, c4 * 4 + tt, :], trp[:])
            # Q heads 1..3
            for h in range(1, G):
                ps = proj([w[:, h * D:(h + 1) * D] for w in wq_sb], D, xsl)
                rope(qt[h][:, sl], ps, sl)

        # ================= phases B+C: attention + o_proj =================
        def attn_head(c, h):
            """Scores/exp/AV/denominator/normalize for (chunk c, head h).
            Returns the normalized ot tile [D, CH] (f32)."""
            dn = dnpool.tile([128, CH], BF16, tag="dn")
            av = psav.tile([D, CH], F32, tag="av")
            et_prev = None
            for t in range(NST):
                sc = pssc.tile([128, CH], F32, tag="sc")
                for hf in range(2):
                    qsl = slice(c * CH + hf * 512, c * CH + (hf + 1) * 512)
                    nc.tensor.matmul(sc[:, hf * 512:(hf + 1) * 512],
                                     _r(kt[:, t * 128:(t + 1) * 128]),
                                     _r(qt[h][:, qsl]), start=True, stop=True)
                et = etpool.tile([128, CH], BF16, tag="et")
                nc.scalar.activation(et[:], sc[:], AF.Exp, scale=SCALE)
                for hf in range(2):
                    nc.tensor.matmul(av[:, hf * 512:(hf + 1) * 512],
                                     vn[:, t, :],
                                     et[:, hf * 512:(hf + 1) * 512],
                                     start=(t == 0), stop=(t == NST - 1))
                if t == 1:
                    nc.vector.tensor_tensor(dn[:], et_prev[:], et[:], OP.add)
                elif t > 1:
                    nc.vector.tensor_tensor(dn[:], dn[:], et[:], OP.add)
                et_prev = et
            # denominator partition-reduce + reciprocal + Pool broadcast
            rc = rcpool.tile([1, CH], BF16, tag="rc")
            for hf in range(2):
                sm = psmx.tile([1, 512], F32, tag="mx", name="sm")
                nc.tensor.matmul(sm[:], ones_sb[:, 0:1],
                                 dn[:, hf * 512:(hf + 1) * 512],
                                 start=True, stop=True)
                with nc.allow_low_precision(reason="bf16 softmax denom recip"):
                    nc.vector.reciprocal(rc[:, hf * 512:(hf + 1) * 512], sm[:])
            bcs = bcspool.tile([128, CH], BF16, tag="bcs")
            nc.gpsimd.partition_broadcast(bcs[:], rc[:])
            ot = otpool.tile([D, CH], F32R, tag="ot")
            nc.vector.tensor_tensor(ot[:], av[:], bcs[:], OP.mult)
            return ot

        def oproj_eo(c, eo, ots):
            """One eo column-group (512 cols of E) of o_proj for chunk c."""
            wot = [wopool.tile([128, 512], F32R, tag="wo", name=f"wo{c}_{eo}_{h}")
                   for h in range(G)]
            for h in range(G):
                nc.sync.dma_start(
                    out=wot[h][:],
                    in_=wo.ap()[h * 128:(h + 1) * 128,
                                eo * 512:(eo + 1) * 512].bitcast(F32R))
            for st in range(CH // 128):
                op = psmx.tile([128, 512], F32, tag="mx", name="op")
                for h in range(G):
                    nc.tensor.matmul(op[:],
                                     _r(ots[h][:, st * 128:(st + 1) * 128]),
                                     _r(wot[h][:]),
                                     start=(h == 0), stop=(h == G - 1))
                ostg = ostgpool.tile([128, 512], F32, tag="ostg")
                if eo % 2 == 0:
                    nc.vector.tensor_copy(ostg[:], op[:])
                else:
                    nc.scalar.copy(ostg[:], op[:])
                nc.sync.dma_start(
                    out=out.ap()[c * CH + st * 128:c * CH + (st + 1) * 128,
                                 eo * 512:(eo + 1) * 512],
                    in_=ostg[:])

        prev_ots = None
        for c in range(NCH):
            cur_ots = []
            for h in range(G):
                cur_ots.append(attn_head(c, h))
                # interleave previous chunk's o_proj into this chunk's PE
                # gaps; front-load (2 eo-groups after each of heads 0/1) so
                # at most 4 prev + 2 cur ot tiles are ever live.
                if prev_ots is not None and h < 2:
                    oproj_eo(c - 1, 2 * h, prev_ots)
                    oproj_eo(c - 1, 2 * h + 1, prev_ots)
            prev_ots = cur_ots
        for eo in range(4):
            oproj_eo(NCH - 1, eo, prev_ots)


def _build():
    nc = bacc.Bacc("TRN2", target_bir_lowering=False, debug=False,
                   num_devices=NCORES)
    xT = nc.dram_tensor("xT", [E, S], F32, kind="ExternalInput")
    wq = nc.dram_tensor("wq", [E, GD], F32, kind="ExternalInput")
    wk = nc.dram_tensor("wk", [E, D], F32, kind="ExternalInput")
    wv = nc.dram_tensor("wv", [E, D], F32, kind="ExternalInput")
    wo = nc.dram_tensor("wo", [GD, E], F32, kind="ExternalInput")
    cosT = nc.dram_tensor("cosT", [D, S], BF16, kind="ExternalInput")
    sinTf = nc.dram_tensor("sinTf", [D, S], BF16, kind="ExternalInput")
    ident = nc.dram_tensor("ident", [128, 128], F32, kind="ExternalInput")
    onesb = nc.dram_tensor("onesb", [128, 128], BF16, kind="ExternalInput")
    out = nc.dram_tensor("out", [S, E], F32, kind="ExternalOutput")
    with tile.TileContext(nc) as tc:
        _emit(nc, tc, xT, wq, wk, wv, wo, cosT, sinTf, ident, onesb, out)
    nc.compile()
    return nc


def _rope_tables():
    inv = 1.0 / (ROPE_BASE ** (np.arange(0, D, 2, dtype=np.float64) / D))
    t = np.arange(S, dtype=np.float64)
    freqs = t[:, None] * inv[None, :]                    # [S, D/2]
    emb = np.concatenate([freqs, freqs], axis=-1)        # [S, D]
    cosT = np.cos(emb).T.astype(ml_dtypes.bfloat16)      # [D, S]
    sinT = np.sin(emb).T
    sinT[: D // 2] *= -1.0                               # fold rotate_half sign
    sinTf = sinT.astype(ml_dtypes.bfloat16)
    return np.ascontiguousarray(cosT), np.ascontiguousarray(sinTf)


_NC = None
LAST_RESULTS = None


def kernel(hidden_states, wq, wk, wv, wo):
    global _NC, LAST_RESULTS
    if _NC is None:
        _NC = _build()
    cosT, sinTf = _rope_tables()
    ident = np.eye(128, dtype=np.float32)
    onesb = np.ones((128, 128), dtype=ml_dtypes.bfloat16)
    hs = np.asarray(hidden_states, dtype=np.float32)
    wq = np.asarray(wq, dtype=np.float32)
    wk = np.asarray(wk, dtype=np.float32)
    wv = np.asarray(wv, dtype=np.float32)
    wo = np.asarray(wo, dtype=np.float32)

    in_maps = []
    for core in range(NCORES):
        b, g = divmod(core, G)
        in_maps.append({
            "xT": np.ascontiguousarray(hs[b].T),
            "wq": np.ascontiguousarray(wq[:, GD * g:GD * (g + 1)]),
            "wk": np.ascontiguousarray(wk[:, D * g:D * (g + 1)]),
            "wv": np.ascontiguousarray(wv[:, D * g:D * (g + 1)]),
            "wo": np.ascontiguousarray(wo[GD * g:GD * (g + 1), :]),
            "cosT": cosT,
            "sinTf": sinTf,
            "ident": ident,
            "onesb": onesb,
        })

    res = run_bass_kernel_spmd(_NC, in_maps, list(range(NCORES)))
    LAST_RESULTS = res
    outs = [np.asarray(res.results[i]["out"], dtype=np.float32)
            for i in range(NCORES)]
    full = np.stack([sum(outs[b * G:(b + 1) * G]) for b in range(B)], axis=0)
    return full.astype(np.float32)
